# revision 18
# baseline (speedup 1.0000x reference)
"""Trainium2 Bass kernel for nn_Attention_16028817948779.

Reference computation (b=4, c=256, heads=8, d=64, h=w=48, n=2304):
  qkv = w_qkv @ x          (1x1 conv)
  q,k,v -> [b, H, d, n];  q,k l2-normalized along n (spatial)
  sim  = (q^T k) * 10;  attn = softmax(sim, axis=-1)
  out  = attn @ v^T -> [b, H, n, d] -> [b, H*d, h, w]
  y    = w_out @ out + b_out

Sharding: 8 cores; core c handles batch c//2, head group (c%2)*4..+4.
Each core computes a partial y over its 4 heads; host sums the two
partials per batch and adds the bias.

Kernel design (v2):
  - All matmuls fp16 (2 cols/cycle at K<=64; 1 col/cycle at K=128).
  - ST in direct form ST[j,i] = k_j . q_i with the full 10/(|q||k|)
    l2norm scale folded per-d-row into q (fp16).  sim range is tiny
    (|sim| < 0.24), so softmax needs no max subtraction.
  - exp split across two engines, one PSUM->SBUF instruction per tile:
      ACT tiles: pt = 2*exp(u)        (exact, Exp with bias=ln2)
      DVE tiles: pt = (u+2)*u         (scalar_tensor_tensor; equals
                 2*e^u - 2 up to O(u^3) ~ 0.07% of the weight scale)
    The DVE tiles' missing constant 2 is injected into the PV PSUM
    accumulation by a rank-1 matmul with lhsT = 2*sum_{j in DVE} v[:,j]
    (computed once per head by a ones-stationary matmul over vt).
  - Softmax denominator from a ones-column appended to V^T (row 64 of
    the PV accumulator); reciprocals computed spread across 128
    partitions via a DMA round-trip (iterative DVE recip on [1,512]
    rows is ~60x more expensive).
  - y written via ACT copy + DMA; host sums core pairs + bias.
"""

import os
import sys

import numpy as np

_TRN_REPO = "/opt/trn_rl_repo"
if _TRN_REPO not in sys.path:
    sys.path.insert(0, _TRN_REPO)

B = 4
C = 256
HEADS = 8
D = 64
N = 2304  # 48*48
HID = HEADS * D  # 512

N_CORES = 8
CI = 2  # c chunks of 128
NCHUNKS = [(0, 512), (512, 512), (1024, 512), (1536, 512), (2048, 256)]
NJ = N // 128  # 18 key chunks of 128

# j-tile exp engine split: ACT exact exp vs (u+1)^2 quadratic whose square
# runs on DVE or GpSimd (i1, the PSUM->SBUF affine, is always DVE)
ACT_JS = (0, 2, 4, 6, 8, 10, 12, 14, 16, 17)
POOL_JS = (1, 3, 5, 9, 11, 15)
DVE_JS = (7, 13)
NONACT_JS = tuple(sorted(POOL_JS + DVE_JS))
LN2 = 0.6931471805599453


def _apply_compat_patches():
    """walrus in this env only accepts ~1 sync wait per instruction, but the
    Tile framework attaches one wait per outstanding proc to a single
    instruction. Split excess waits onto EventSemaphore instructions at the
    BIR-JSON level (Bass.to_json_bytes is the serialization choke point for
    both the native and the axon/PJRT compile paths)."""
    import json

    import concourse.bass as bass

    if getattr(bass.Bass.to_json_bytes, "_waitsplit", False):
        return

    MAXW = 1
    _orig = bass.Bass.to_json_bytes

    def _split_waits(raw):
        m = json.loads(raw)
        ctr = 0
        changed = False
        for f in m.get("functions", []):
            for blk in f.get("blocks", []):
                new_insts = []
                for ins in blk.get("instructions", []):
                    si = ins.get("sync_info")
                    waits = (si or {}).get("on_wait") or []
                    if len(waits) > MAXW:
                        changed = True
                        for w in waits[:-MAXW]:
                            ctr += 1
                            new_insts.append(
                                {
                                    "debug": ins.get("debug", 0),
                                    "engine": ins["engine"],
                                    "ins": [],
                                    "outs": [],
                                    "name": f"waitsplit_{ctr}",
                                    "opcode": "EventSemaphore",
                                    "sync_info": {"on_update": [], "on_wait": [w]},
                                }
                            )
                        si["on_wait"] = waits[-MAXW:]
                    new_insts.append(ins)
                blk["instructions"] = new_insts
        return json.dumps(m).encode() if changed else raw

    def _patched(self):
        return _split_waits(_orig(self))

    _patched._waitsplit = True
    bass.Bass.to_json_bytes = _patched


def build_kernel():
    import concourse.bass as bass
    import concourse.mybir as mybir
    import concourse.tile as tile

    _apply_compat_patches()

    f32 = mybir.dt.float32
    f16 = mybir.dt.float16
    Exp = mybir.ActivationFunctionType.Exp
    Ln = mybir.ActivationFunctionType.Ln
    Copy = mybir.ActivationFunctionType.Copy
    mult = mybir.AluOpType.mult
    add = mybir.AluOpType.add

    nc = bass.Bass()
    x_d = nc.dram_tensor("x", [128, CI, N], f16, kind="ExternalInput")
    wqT_d = nc.dram_tensor("wqT", [128, CI, 256], f16, kind="ExternalInput")
    wkT_d = nc.dram_tensor("wkT", [128, CI, 256], f16, kind="ExternalInput")
    wvT_d = nc.dram_tensor("wvT", [128, CI, 256], f16, kind="ExternalInput")
    woutT_d = nc.dram_tensor("woutT", [64, 4, 256], f16, kind="ExternalInput")
    y_d = nc.dram_tensor("y", [C, N], f32, kind="ExternalOutput")

    with tile.TileContext(nc) as tc:
        with (
            tc.tile_pool(name="persist", bufs=1) as pp,
            tc.tile_pool(name="pt", bufs=4) as ptp,
            tc.tile_pool(name="misc", bufs=2) as mp,
            tc.tile_pool(name="dram", bufs=2, space="DRAM") as dp,
            tc.tile_pool(name="ps_st", bufs=2, space="PSUM") as ps_st,
            tc.tile_pool(name="ps_pv", bufs=2, space="PSUM") as ps_pv,
        ):
            # ---- load inputs ----
            x_sb = pp.tile([128, CI, N], f16)
            for ci in range(CI):
                for ns, nl in NCHUNKS:
                    nc.sync.dma_start(
                        out=x_sb[:, ci, ns : ns + nl],
                        in_=x_d[:, ci, ns : ns + nl],
                    )
            wq_sb = pp.tile([128, CI, 256], f16)
            wk_sb = pp.tile([128, CI, 256], f16)
            wv_sb = pp.tile([128, CI, 256], f16)
            for w_sb, w_d in ((wq_sb, wqT_d), (wk_sb, wkT_d), (wv_sb, wvT_d)):
                nc.sync.dma_start(out=w_sb[:], in_=w_d[:])
            wo_sb = pp.tile([64, 4, 256], f16)
            nc.sync.dma_start(out=wo_sb[:], in_=woutT_d[:])

            ones_f16 = pp.tile([128, 1], f16)
            nc.vector.memset(ones_f16[:], 1.0)
            ones_row = pp.tile([1, 512], f16)
            nc.vector.memset(ones_row[:], 1.0)
            ln2_c = pp.tile([128, 1], f32)
            nc.vector.memset(ln2_c[:], LN2)
            zero_sb = pp.tile([128, 66], f16)
            nc.vector.memset(zero_sb[:], 0.0)

            # PE warm-up during the initial DMA wait so the clock gate is
            # fully open when real matmuls arrive.
            warm_sb = pp.tile([128, 512], f16)
            nc.vector.memset(warm_sb[:], 1.0)
            warm_ps = ps_st.tile([128, 2, 512], f32, tag="st", name="warm_ps")
            for wi in range(32):
                nc.tensor.matmul(
                    warm_ps[:, 0, :],
                    lhsT=warm_sb[:, 0:128],
                    rhs=warm_sb[:],
                    start=(wi == 0),
                    stop=(wi == 31),
                )
            nc.vector.tensor_copy(warm_sb[:, 0:16], warm_ps[:, 0, 0:16])

            # ---- V projection -> vt [128, j, head, 64 cols + ones col] ----
            vt = pp.tile([128, NJ, 4, 66], f16)
            nc.vector.memset(vt[:, :, :, 64:65], 1.0)
            for j in range(NJ):
                ps = ps_pv.tile([128, 512], f32, tag="pv", name="v_ps")
                for ci in range(CI):
                    nc.tensor.matmul(
                        ps[:, 0:256],
                        lhsT=x_sb[:, ci, j * 128 : (j + 1) * 128],
                        rhs=wv_sb[:, ci, :],
                        start=(ci == 0),
                        stop=(ci == CI - 1),
                    )
                eng = nc.scalar.copy if j % 2 == 0 else nc.vector.tensor_copy
                eng(
                    vt[:, j, :, 0:64],
                    ps[:, 0:256].rearrange("p (h d) -> p h d", h=4),
                )

            # ---- vsum for the quadratic tiles' missing constant (+1) ----
            # vs_ps[0, (h,c)] = sum_{j in NONACT_JS} vt[p, j, h, c]
            vs_ps = ps_pv.tile([1, 264], f32, tag="pv", name="vs_ps")
            for idx, j in enumerate(NONACT_JS):
                nc.tensor.matmul(
                    vs_ps[:],
                    lhsT=ones_f16[:, 0:1],
                    rhs=vt[:, j, :, :].rearrange("p h c -> p (h c)"),
                    start=(idx == 0),
                    stop=(idx == len(NONACT_JS) - 1),
                )
            vsum_sb = pp.tile([1, 4, 66], f16)
            nc.scalar.activation(
                vsum_sb.rearrange("p h c -> p (h c)"), vs_ps[:], Copy
            )

            # ---- QK projection -> qf, kf fp16; ssq from batched ACT squares ----
            qf = pp.tile([128, CI, N], f16)
            kf = pp.tile([128, CI, N], f16)
            ssq = mp.tile([128, 2, 2], f32, tag="ssq")
            scratch = pp.tile([128, N], f16)
            for ti, (dst, w_sb) in enumerate(((qf, wq_sb), (kf, wk_sb))):
                for oc in range(2):
                    for nci, (ns, nl) in enumerate(NCHUNKS):
                        ps = ps_pv.tile([128, 512], f32, tag="pv", name="qk_ps")
                        for ci in range(CI):
                            nc.tensor.matmul(
                                ps[:, :nl],
                                lhsT=w_sb[:, ci, oc * 128 : (oc + 1) * 128],
                                rhs=x_sb[:, ci, ns : ns + nl],
                                start=(ci == 0),
                                stop=(ci == CI - 1),
                            )
                        nc.vector.tensor_copy(dst[:, oc, ns : ns + nl], ps[:, :nl])
                    nc.scalar.activation(
                        scratch[:],
                        dst[:, oc, :],
                        mybir.ActivationFunctionType.Square,
                        accum_out=ssq[:, ti, oc : oc + 1],
                    )

            # ---- gamma = 10/sqrt(ssq_q*ssq_k) per d-row, folded into q ----
            gam = mp.tile([128, 2], f32, tag="gam")
            nc.vector.tensor_tensor(gam[:], ssq[:, 0, :], ssq[:, 1, :], mult)
            # 10/sqrt(x) = exp(-0.5*ln(x) + ln(10)); Ln and Exp share a table
            nc.scalar.activation(gam[:], gam[:], Ln)
            ln10 = mp.tile([128, 1], f32, tag="ln10")
            nc.vector.memset(ln10[:], 2.302585092994046)
            nc.scalar.activation(gam[:], gam[:], Exp, bias=ln10[:], scale=-0.5)
            qs = pp.tile([128, CI, N], f16)
            with nc.allow_low_precision(reason="q scale written as fp16"):
                for oc in range(2):
                    for ns, nl in NCHUNKS:
                        nc.vector.tensor_scalar_mul(
                            qs[:, oc, ns : ns + nl],
                            qf[:, oc, ns : ns + nl],
                            gam[:, oc : oc + 1],
                        )

            # ---- attention ----
            outT = pp.tile([64, 4, N], f16)
            pend = []  # (p, nci, pv, bc) awaiting the deferred DVE mult

            def emit_proj(ns, il):
                for oc_ in range(2):
                    yps = ps_st.tile([128, 2, 512], f32, tag="st", name="yps")
                    for h in range(4):
                        nc.tensor.matmul(
                            yps[:, 0, :il],
                            lhsT=wo_sb[:, h, oc_ * 128 : (oc_ + 1) * 128],
                            rhs=outT[:, h, ns : ns + il],
                            start=(h == 0),
                            stop=(h == 3),
                        )
                    y_sb = mp.tile([128, 512], f32, tag="ysb", name="y_sb")
                    nc.scalar.copy(y_sb[:, :il], yps[:, 0, :il])
                    nc.sync.dma_start(
                        out=y_d[oc_ * 128 : (oc_ + 1) * 128, ns : ns + il],
                        in_=y_sb[:, :il],
                    )

            def norm_a(nci_, pv_):
                """den extract -> spread recip -> bc broadcast (ACT + DMA)."""
                ns_, il_ = NCHUNKS[nci_]
                g = il_ // 128  # spread cols per slot (4 or 2)
                den = mp.tile([1, 2, 512], f32, tag="den", name="den", bufs=3)
                nc.scalar.copy(den[:, :, :il_], pv_[64:65, :, :il_])
                gg = 2 * g  # total spread cols (8 or 4)
                dden = dp.tile([1024], f32, tag="dden", name="dden")
                nc.sync.dma_start(out=dden[0 : 2 * il_], in_=den[0:1, :, :il_])
                spread = mp.tile([128, 8], f32, tag="spr", name="spr", bufs=3)
                nc.sync.dma_start(
                    out=spread[:, 0:gg],
                    in_=dden[0 : 2 * il_].rearrange("(c p) -> p c", p=128),
                )
                rsp = mp.tile([128, 8], f32, tag="rsp", name="rsp", bufs=3)
                nc.vector.reciprocal(rsp[:, 0:gg], spread[:, 0:gg])
                rden_d = dp.tile([1024], f32, tag="rden", name="rden")
                nc.sync.dma_start(
                    out=rden_d[0 : 2 * il_].rearrange("(c p) -> p c", p=128),
                    in_=rsp[:, 0:gg],
                )
                bc = mp.tile([64, 2, 512], f32, tag="bc", name="bc", bufs=3)
                for slot in range(2):
                    nc.sync.dma_start(
                        out=bc[:, slot, :il_],
                        in_=rden_d[slot * il_ : (slot + 1) * il_]
                        .rearrange("(a b) -> a b", a=1)
                        .to_broadcast((64, il_)),
                    )
                return bc

            def norm_b(p_, nci_, pv_, bc_):
                """deferred DVE multiply: outT = pv * (1/den)."""
                ns_, il_ = NCHUNKS[nci_]
                nc.vector.tensor_tensor(
                    outT[:, 2 * p_ : 2 * p_ + 2, ns_ : ns_ + il_],
                    pv_[0:64, :, :il_],
                    bc_[:, :, :il_],
                    mult,
                )

            for p in range(2):
                hA, hB = 2 * p, 2 * p + 1
                for nci, (ns, il) in enumerate(NCHUNKS):
                    pv = ps_pv.tile([65, 2, 512], f32, tag="pv", name="pv")
                    # constant-2 injection for the DVE-share tiles
                    for slot, h in ((0, hA), (1, hB)):
                        nc.tensor.matmul(
                            pv[:, slot, :il],
                            lhsT=vsum_sb[:, h, 0:65],
                            rhs=ones_row[:, :il],
                            start=True,
                            stop=False,
                        )
                    pvq = []  # (j, pt) exp outputs awaiting PV emission

                    def emit_pv(j_, pt_, last):
                        for slot, h in ((0, hA), (1, hB)):
                            nc.tensor.matmul(
                                pv[:, slot, :il],
                                lhsT=vt[:, j_, h, 0:65],
                                rhs=pt_[:, slot, :il],
                                start=False,
                                stop=last,
                            )

                    for j in range(NJ):
                        # dependency-free +0 accumulations keep the PE busy
                        # through exp-wait bubbles so the clock stays ramped
                        for _ in range(2):
                            nc.tensor.matmul(
                                pv[:, 0, :il],
                                lhsT=zero_sb[:, 0:65],
                                rhs=warm_sb[:, :il],
                                start=False,
                                stop=False,
                            )
                        st = ps_st.tile([128, 2, 512], f32, tag="st", name="st")
                        nc.tensor.matmul(
                            st[:, 0, :il],
                            lhsT=kf[0:64, p, j * 128 : (j + 1) * 128],
                            rhs=qs[0:64, p, ns : ns + il],
                        )
                        nc.tensor.matmul(
                            st[:, 1, :il],
                            lhsT=kf[64:128, p, j * 128 : (j + 1) * 128],
                            rhs=qs[64:128, p, ns : ns + il],
                        )
                        pt = ptp.tile([128, 2, 512], f16, tag="pt", name="pt", bufs=6)
                        if j in ACT_JS:
                            nc.scalar.activation(
                                pt[:, :, :il], st[:, :, :il], Exp, bias=ln2_c[:]
                            )
                        else:
                            w16 = ptp.tile(
                                [128, 2, 512], f16, tag="w16", name="w16"
                            )
                            nc.vector.tensor_scalar(
                                out=w16[:, :, :il],
                                in0=st[:, :, :il],
                                scalar1=1.0,
                                scalar2=None,
                                op0=add,
                            )
                            sq_eng = (
                                nc.gpsimd if j in POOL_JS else nc.vector
                            )
                            sq_eng.tensor_tensor(
                                pt[:, :, :il], w16[:, :, :il], w16[:, :, :il], mult
                            )
                        # lag PV emission so a slow exp tile never stalls the
                        # PE queue (accumulation adds commute within a group)
                        pvq.append((j, pt))
                        if len(pvq) > 2:
                            emit_pv(*pvq.pop(0), False)
                        # overlap: previous chunks' deferred work
                        if j == 2 and pend:
                            norm_b(*pend.pop(0))
                        if j == 8 and p == 1 and nci > 0:
                            emit_proj(*NCHUNKS[nci - 1])
                    while pvq:
                        jj, ptt = pvq.pop(0)
                        emit_pv(jj, ptt, not pvq)
                    bc = norm_a(nci, pv)
                    pend.append((p, nci, pv, bc))

            while pend:
                norm_b(*pend.pop(0))
            emit_proj(*NCHUNKS[-1])

    return nc


_NC_CACHE = None


def kernel(x, w_qkv, w_out, b_out):
    global _NC_CACHE
    from concourse.bass_utils import run_bass_kernel_spmd

    x = np.ascontiguousarray(x, dtype=np.float32)
    w_qkv = np.asarray(w_qkv, dtype=np.float32)
    w_out = np.asarray(w_out, dtype=np.float32)
    b_out = np.asarray(b_out, dtype=np.float32)

    b, c, h, w = x.shape
    assert (b, c, h, w) == (B, C, 48, 48)
    x_bn = x.reshape(B, C, N)

    wq, wk, wv = w_qkv[0:HID], w_qkv[HID : 2 * HID], w_qkv[2 * HID : 3 * HID]
    w_outT = np.ascontiguousarray(w_out.T)  # [HID, C]

    in_maps = []
    for core in range(N_CORES):
        bb, g = core // 2, core % 2
        rows = slice(g * 256, g * 256 + 256)
        woutT_c = np.ascontiguousarray(
            w_outT[rows].reshape(4, 64, 256).transpose(1, 0, 2)
        ).astype(np.float16)
        x_c = np.ascontiguousarray(
            x_bn[bb].reshape(CI, 128, N).transpose(1, 0, 2)
        ).astype(np.float16)
        in_maps.append(
            {
                "x": x_c,
                "wqT": np.ascontiguousarray(
                    wq[rows].T.reshape(CI, 128, 256).transpose(1, 0, 2)
                ).astype(np.float16),
                "wkT": np.ascontiguousarray(
                    wk[rows].T.reshape(CI, 128, 256).transpose(1, 0, 2)
                ).astype(np.float16),
                "wvT": np.ascontiguousarray(
                    wv[rows].T.reshape(CI, 128, 256).transpose(1, 0, 2)
                ).astype(np.float16),
                "woutT": woutT_c,
            }
        )

    if _NC_CACHE is None:
        _NC_CACHE = build_kernel()
    nc = _NC_CACHE

    trace = bool(int(os.environ.get("KERNEL_TRACE", "0")))
    trace_cores = [
        int(cc) for cc in os.environ.get("KERNEL_TRACE_CORES", "0").split(",")
    ]
    res = run_bass_kernel_spmd(
        nc,
        in_maps,
        core_ids=list(range(N_CORES)),
        trace=trace,
        trace_cores=trace_cores if trace else None,
    )
    kernel.last_result = res

    y = np.empty((B, C, N), dtype=np.float32)
    for bb in range(B):
        y[bb] = (
            res.results[2 * bb]["y"]
            + res.results[2 * bb + 1]["y"]
            + b_out[:, None]
        )
    return y.reshape(B, C, 48, 48)
